# revision 25
# baseline (speedup 1.0000x reference)
"""Trainium2 Bass kernel for nn_Channel_Wise_DiffLoss.

Reference computation (P = 16384 pixels, C = 2048 columns = B*C_ch):
    x1 = input1.reshape(P, C);  x2 = input2.reshape(P, C)
    n_i[c] = sqrt(sum_p x_i[p,c]^2)          (per-column L2 norm)
    x_in = x_i / (n_i + 1e-6)
    out  = mean(x1n^T @ x2n) ** 2

Algebraic rewrite (no Gram matrix needed):
    mean(gram) = (1/C^2) * sum_p s1[p] * s2[p]
    where s_i[p] = sum_c x_i[p,c] * r_i[c],  r_i[c] = 1/(n_i[c] + 1e-6)

With 16384-element Gaussian columns, n ~ 128 >> 1e-6, and (n + 1e-6)
rounds to n exactly in fp32, so r = rsqrt(ssq) is exact.

Sharding: columns across the 8 cores (256 columns each). Column norms are
then fully core-local (each core holds the full pixel extent of its
columns) -> no collectives. Each core returns its partial s1/s2 vectors
(sum over its 256 columns); the host adds the 8 partials and does the
final tiny dot product.

Per-core pipeline (v2 — single pass over HBM, every engine under the
DMA roofline of ~94 us for the 32 MiB shard):
  - 32 chunks of [128 cols, 2048 pixels] fp32 (1 MiB each), DMA'd on BOTH
    HWDGE rings (sync + scalar engine issue) to aggregate queue bandwidth.
  - DVE converts each chunk fp32 -> bf16 (2x_2P mode, ~1.1 us/chunk) into
    a persistent per-block [128, 16384] bf16 tile.
  - Column sum-of-squares per chunk via ACT Square+accumulator (ScalarE)
    or scalar_tensor_tensor+accum (GPSIMD), split to balance engines.
  - Per block: reduce partials, sqrt (ACT table, 2ULP), DVE reciprocal +
    one Newton step, replicate to bf16 weights [128, 32].
  - TensorE: 32 matmuls [128,512] bf16 per (input, block) (1 col/cycle).
    PSUM bases are limited to {0,32,64}, so chunks land in 4-round
    ping-pong tiles of 2 bases x 4 banks (the baseline geometry); rows
    31:33 of each round drain to SBUF (split DVE/ScalarE) and DMA out.
    The host adds the two block partials per input.
"""

import numpy as np

import concourse.bass as bass
import concourse.mybir as mybir
from concourse import tile
from concourse import bass_utils

P_TOT = 16384  # pixels (H*W)
C_TOT = 2048  # columns (B*C)
N_CORES = 8
C_CORE = C_TOT // N_CORES  # 256 columns per core
CB = C_CORE // 128  # 2 column blocks of 128 partitions
NCHUNK = 8  # DMA/compute chunks per block (1 MiB each)
CHUNK_F = P_TOT // NCHUNK  # 2048 pixels per chunk
MMN = 512  # matmul moving free size (one PSUM bank of fp32)
NMM = P_TOT // MMN  # 32 matmul chunks per block

_F32 = mybir.dt.float32
_BF16 = mybir.dt.bfloat16

# Per-block chunk roles, balanced from the v3 trace (DVE was 98.8us —
# scalar_tensor_tensor runs at 1x, 2285ns, and stalled the stage
# pipeline; tensor_tensor_reduce mis-encodes - "ISA wrong length"):
#   - converts fp32->bf16: chunks 0-5 GPSIMD tensor_copy (~2.9us, the
#     engine is otherwise idle), chunks 6-7 DVE CAST (~1.2us, keeps the
#     block tail short)
#   - ssq: chunks 0-4 ScalarE ACT Square+accum from the fp32 stage
#     (~1.8us, parallel with the convert), chunks 5-7 DVE
#     scalar_tensor_tensor on the bf16 copy
#   - PSUM round drains alternate ScalarE (ACT ~1.96us) / DVE (~2.7us)
SSQ_VECTOR = (5, 6, 7)  # chunks per block with DVE ssq (rest ScalarE)
CONV_VECTOR = (6, 7)  # chunks per block with DVE convert (rest GPSIMD)

_cache = {}

# Results of the last device run (BassKernelResults); the test harness
# reads exec_time_ns off this after calling kernel(..., _trace=True).
LAST_RESULTS = None


def _emit_core_kernel(nc, tc, ctx, xts, s_out):
    """xts = [x1t, x2t] DRAM APs [C_CORE, P_TOT]; s_out [2, 4, NMM//4, MMN]."""
    stage = ctx.enter_context(tc.tile_pool(name="stage", bufs=6))
    x16p = ctx.enter_context(tc.tile_pool(name="x16", bufs=2))
    stat = ctx.enter_context(tc.tile_pool(name="stat", bufs=8))
    const = ctx.enter_context(tc.tile_pool(name="const", bufs=1))
    sqs = ctx.enter_context(tc.tile_pool(name="sqs", bufs=2))
    sqv = ctx.enter_context(tc.tile_pool(name="sqv", bufs=2))
    psum = ctx.enter_context(tc.tile_pool(name="psum", bufs=2, space="PSUM"))
    stp = ctx.enter_context(tc.tile_pool(name="stp", bufs=3))

    ones = const.tile([128, 32], _F32, tag="ones")
    nc.vector.memset(ones[:], 1.0)

    # Warm-up: trigger ACT table loads at kernel start so they don't land
    # on the pipelined squares/sqrts.
    warm = const.tile([128, 1], _F32, tag="warm")
    nc.scalar.activation(
        warm[:], ones[:, 0:1], mybir.ActivationFunctionType.Square
    )
    nc.scalar.sqrt(warm[:], warm[:])

    cj = 0  # global chunk id
    prev_pts = []  # previous block's PSUM round tiles (for PE warm-ups)
    for i, xt in enumerate(xts):
        for b in range(CB):
            xb16 = x16p.tile([128, P_TOT], _BF16, tag="xb16")
            parts = stat.tile([128, NCHUNK], _F32, tag="parts")
            for j in range(NCHUNK):
                stg = stage.tile([128, CHUNK_F], _F32, tag="stg")
                dma_eng = nc.scalar if cj % 3 == 1 else nc.sync
                dma_eng.dma_start(
                    stg[:], xt[b * 128 : (b + 1) * 128, bass.ts(j, CHUNK_F)]
                )
                # PE warm-up: one tiny matmul per chunk, gated on the
                # chunk's DMA (reads one staged cell), into a dead cell
                # (row 64, never drained) of a prior round tile.  Keeps
                # the HAM activity window busy so the real matmul bursts
                # run at 2.4 GHz instead of ramping from 1.2.
                if prev_pts:
                    wt = prev_pts[-2 + (j % 2)]
                    nc.tensor.matmul(
                        wt[64:65, 0, 0:1], ones[0:1, 0:1], stg[0:1, 0:1],
                        start=True, stop=True, skip_group_check=True,
                    )
                # fp32 -> bf16 for the TensorE stream
                x16c = xb16[:, bass.ts(j, CHUNK_F)]
                if j in CONV_VECTOR:
                    nc.vector.tensor_copy(x16c, stg[:])
                else:
                    nc.gpsimd.tensor_copy(x16c, stg[:])
                # column sum-of-squares partial for this chunk
                if j in SSQ_VECTOR:
                    sq = sqv.tile([128, CHUNK_F], _BF16, tag="sq")
                    nc.vector.scalar_tensor_tensor(
                        sq[:],
                        x16c,
                        1.0,
                        x16c,
                        op0=mybir.AluOpType.mult,
                        op1=mybir.AluOpType.mult,
                        accum_out=parts[:, j : j + 1],
                    )
                else:
                    sq = sqs.tile([128, CHUNK_F], _F32, tag="sq")
                    nc.scalar.activation(
                        sq[:],
                        stg[:],
                        mybir.ActivationFunctionType.Square,
                        accum_out=parts[:, j : j + 1],
                    )
                cj += 1

            # r = 1/sqrt(ssq): ACT-table sqrt (<=2ULP) + DVE reciprocal +
            # one Newton step on the reciprocal (y <- y*(2 - n*y)).
            ssq = stat.tile([128, 1], _F32, tag="ssq")
            nc.vector.reduce_sum(ssq[:], parts[:], axis=mybir.AxisListType.X)
            n_ = stat.tile([128, 1], _F32, tag="n_")
            nc.scalar.sqrt(n_[:], ssq[:])
            y = stat.tile([128, 1], _F32, tag="y")
            nc.vector.reciprocal(y[:], n_[:])
            t0 = stat.tile([128, 1], _F32, tag="t0")
            nc.vector.tensor_mul(t0[:], y[:], n_[:])
            t1 = stat.tile([128, 1], _F32, tag="t1")
            nc.vector.tensor_scalar(
                t1[:], t0[:], -1.0, 2.0,
                op0=mybir.AluOpType.mult, op1=mybir.AluOpType.add,
            )
            y2 = stat.tile([128, 1], _F32, tag="y2")
            nc.vector.tensor_mul(y2[:], y[:], t1[:])
            # replicate r across 32 stationary weight columns, in bf16
            yb = stat.tile([128, 32], _BF16, tag="yb")
            nc.vector.tensor_scalar(
                yb[:], ones[:], y2[:, 0:1], None, op0=mybir.AluOpType.mult
            )

            # s contributions: contract the 128 columns (partitions) via
            # matmul into 4-round ping-pong PSUM tiles (2 bases x 4
            # banks, 8 pixel chunks each).  Rows 0-31 replicate the
            # base-0 chunk, row 32 is the base-32 chunk's first replica,
            # so one [33, 2048] copy (cost = free size only) captures a
            # whole round; drains run on ScalarE and chase the matmul
            # stream.  Out rows 31:33 per round; host adds the
            # per-block partials.
            cur_pts = []
            for r in range(4):
                pt = psum.tile([128, 4, MMN], _F32, tag="pt")
                cur_pts.append(pt)
                for base_idx in range(2):
                    for bank in range(4):
                        j2 = r * 8 + base_idx * 4 + bank
                        nc.tensor.matmul(
                            pt[32 * base_idx : 32 * base_idx + 32, bank, :],
                            yb[:],
                            xb16[:, bass.ts(j2, MMN)],
                            start=True,
                            stop=True,
                        )
                st = stp.tile([33, 4 * MMN], _F32, tag="st")
                if r % 2 == 0:
                    nc.scalar.copy(st[:], pt[0:33, :, :])
                else:
                    nc.vector.tensor_copy(st[:], pt[0:33, :, :])
                nc.sync.dma_start(s_out[i, b, r], st[31:33, :])
            prev_pts = cur_pts


def _hoist_excess_waits(nc):
    """Walrus rejects instructions whose encodings lack room for multiple
    semaphore waits (Activation/LoadWeights/DMA-direct2d allow just one).
    Hoist all-but-one wait of any instruction into standalone
    InstEventSemaphore waits on the same engine queue — semantically
    identical (the queue blocks at the event-sem instead)."""
    cnt = 0
    for f in nc.m.functions:
        for blk in f.blocks:
            insts = blk.instructions
            out = []
            changed = False
            for inst in insts:
                si = getattr(inst, "sync_info", None)
                waits = list(si.on_wait) if si is not None and si.on_wait else []
                if len(waits) > 1:
                    for w in waits[:-1]:
                        ev = mybir.InstEventSemaphore(
                            name=f"I-hoistw-{cnt}", ins=[], outs=[]
                        )
                        cnt += 1
                        ev.engine = inst.engine
                        ev.sync_info = mybir.SyncInfo(on_wait=[w], on_update=[])
                        out.append(ev)
                    inst.sync_info = mybir.SyncInfo(
                        on_wait=[waits[-1]],
                        on_update=list(si.on_update or []),
                    )
                    changed = True
                out.append(inst)
            if changed:
                insts[:] = out
    return cnt


def _build(hoist=True):
    # hoist=False is for CoreSim-based validation only (the simulator
    # can't ingest the raw-inserted event-sem instructions).
    key = ("nc", hoist)
    if key in _cache:
        return _cache[key]
    nc = bass.Bass("TRN2", target_bir_lowering=False, debug=False,
                   num_devices=N_CORES)
    x1t = nc.dram_tensor("x1t", [C_CORE, P_TOT], _F32, kind="ExternalInput").ap()
    x2t = nc.dram_tensor("x2t", [C_CORE, P_TOT], _F32, kind="ExternalInput").ap()
    s_out = nc.dram_tensor(
        "s_out", [2, CB, 4, 2, C_TOT], _F32, kind="ExternalOutput"
    ).ap()
    from contextlib import ExitStack

    with tile.TileContext(nc) as tc:
        with ExitStack() as ctx:
            _emit_core_kernel(nc, tc, ctx, [x1t, x2t], s_out)
    if hoist:
        _hoist_excess_waits(nc)
    _cache[key] = nc
    return nc


def _shard_inputs(input1, input2):
    """Column-shard + transpose: core k gets x[:, k*256:(k+1)*256].T
    contiguous [C_CORE, P_TOT] so DMA rows are 64 KiB contiguous."""
    in_maps = [{} for _ in range(N_CORES)]
    for name, arr in (("x1t", input1), ("x2t", input2)):
        x = np.ascontiguousarray(np.asarray(arr, dtype=np.float32)).reshape(
            P_TOT, C_TOT
        )
        xs = np.ascontiguousarray(x.reshape(P_TOT, N_CORES, C_CORE).transpose(1, 2, 0))
        for k in range(N_CORES):
            in_maps[k][name] = xs[k]
    return in_maps


def _unscramble(s_core):
    """s_core: [CB, 4, 2, 2048] for one input. Pixel index is
    (r*8 + base_idx*4 + bank)*512 + n = row-major flatten of
    [r, base_idx, bank, n]; block partials sum."""
    return s_core.astype(np.float64).sum(axis=0).reshape(P_TOT)


def kernel(input1, input2, _trace=False):
    global LAST_RESULTS
    nc = _build()
    in_maps = _shard_inputs(input1, input2)
    res = bass_utils.run_bass_kernel_spmd(
        nc, in_maps, core_ids=list(range(N_CORES)), trace=_trace,
    )
    LAST_RESULTS = res
    s1 = np.zeros(P_TOT, dtype=np.float64)
    s2 = np.zeros(P_TOT, dtype=np.float64)
    for r in res.results:
        so = r["s_out"]  # [2, CB, 4, 2, 2048]
        s1 += _unscramble(so[0])
        s2 += _unscramble(so[1])
    dot = float(np.dot(s1, s2))
    mean = dot / (C_TOT * C_TOT)
    return np.array(mean * mean, dtype=np.float32)


# revision 27
# speedup vs baseline: 1.8415x; 1.8415x over previous
"""Trainium2 Bass kernel for nn_Channel_Wise_DiffLoss.

Reference computation (P = 16384 pixels, C = 2048 columns = B*C_ch):
    x1 = input1.reshape(P, C);  x2 = input2.reshape(P, C)
    n_i[c] = sqrt(sum_p x_i[p,c]^2)          (per-column L2 norm)
    x_in = x_i / (n_i + 1e-6)
    out  = mean(x1n^T @ x2n) ** 2

Algebraic rewrite (no Gram matrix needed):
    mean(gram) = (1/C^2) * sum_p s1[p] * s2[p]
    where s_i[p] = sum_c x_i[p,c] * r_i[c],  r_i[c] = 1/(n_i[c] + 1e-6)

With 16384-element Gaussian columns, n ~ 128 >> 1e-6, and (n + 1e-6)
rounds to n exactly in fp32, so r = rsqrt(ssq) is exact.

Sharding: columns across the 8 cores (256 columns each). Column norms are
then fully core-local (each core holds the full pixel extent of its
columns) -> no collectives. Each core returns its partial s1/s2 vectors
(sum over its 256 columns); the host adds the 8 partials and does the
final tiny dot product.

Per-core pipeline (v2 — single pass over HBM, every engine under the
DMA roofline of ~94 us for the 32 MiB shard):
  - 32 chunks of [128 cols, 2048 pixels] fp32 (1 MiB each), DMA'd on BOTH
    HWDGE rings (sync + scalar engine issue) to aggregate queue bandwidth.
  - DVE converts each chunk fp32 -> bf16 (2x_2P mode, ~1.1 us/chunk) into
    a persistent per-block [128, 16384] bf16 tile.
  - Column sum-of-squares per chunk via ACT Square+accumulator (ScalarE)
    or scalar_tensor_tensor+accum (GPSIMD), split to balance engines.
  - Per block: reduce partials, sqrt (ACT table, 2ULP), DVE reciprocal +
    one Newton step, replicate to bf16 weights [128, 32].
  - TensorE: 32 matmuls [128,512] bf16 per (input, block) (1 col/cycle).
    PSUM bases are limited to {0,32,64}, so chunks land in 4-round
    ping-pong tiles of 2 bases x 4 banks (the baseline geometry); rows
    31:33 of each round drain to SBUF (split DVE/ScalarE) and DMA out.
    The host adds the two block partials per input.
"""

import numpy as np

import concourse.bass as bass
import concourse.mybir as mybir
from concourse import tile
from concourse import bass_utils

P_TOT = 16384  # pixels (H*W)
C_TOT = 2048  # columns (B*C)
N_CORES = 8
C_CORE = C_TOT // N_CORES  # 256 columns per core
CB = C_CORE // 128  # 2 column blocks of 128 partitions
NCHUNK = 8  # DMA/compute chunks per block (1 MiB each)
CHUNK_F = P_TOT // NCHUNK  # 2048 pixels per chunk
MMN = 512  # matmul moving free size (one PSUM bank of fp32)
NMM = P_TOT // MMN  # 32 matmul chunks per block

_F32 = mybir.dt.float32
_BF16 = mybir.dt.bfloat16

# Per-block chunk roles, balanced from the v3 trace (DVE was 98.8us —
# scalar_tensor_tensor runs at 1x, 2285ns, and stalled the stage
# pipeline; tensor_tensor_reduce mis-encodes - "ISA wrong length"):
#   - converts fp32->bf16: all DVE CAST (~1.2us 2x_2P).  GPSIMD CAST
#     measured 7.6us/chunk AND its shared SBUF port throttled concurrent
#     DVE ops 2.4x — keep GPSIMD idle.
#   - ssq: all ScalarE ACT Square+accum from the fp32 stage (~1.9us +
#     0.3us accumulator read, parallel with the convert)
#   - all PSUM round drains on DVE (~2.3us)
# Balance: ScalarE ~87us, DVE ~88us, under the ~102us DMA span.
SSQ_VECTOR = ()  # chunks per block with DVE ssq (rest ScalarE)
CONV_VECTOR = tuple(range(NCHUNK))  # chunks with DVE convert (rest GPSIMD)

_cache = {}

# Results of the last device run (BassKernelResults); the test harness
# reads exec_time_ns off this after calling kernel(..., _trace=True).
LAST_RESULTS = None


def _emit_core_kernel(nc, tc, ctx, xts, s_out):
    """xts = [x1t, x2t] DRAM APs [C_CORE, P_TOT]; s_out [2, 4, NMM//4, MMN]."""
    stage = ctx.enter_context(tc.tile_pool(name="stage", bufs=6))
    x16p = ctx.enter_context(tc.tile_pool(name="x16", bufs=2))
    stat = ctx.enter_context(tc.tile_pool(name="stat", bufs=8))
    const = ctx.enter_context(tc.tile_pool(name="const", bufs=1))
    sqs = ctx.enter_context(tc.tile_pool(name="sqs", bufs=2))
    sqv = ctx.enter_context(tc.tile_pool(name="sqv", bufs=2))
    psum = ctx.enter_context(tc.tile_pool(name="psum", bufs=2, space="PSUM"))
    stp = ctx.enter_context(tc.tile_pool(name="stp", bufs=3))

    ones = const.tile([128, 32], _F32, tag="ones")
    nc.vector.memset(ones[:], 1.0)

    # Warm-up: trigger ACT table loads at kernel start so they don't land
    # on the pipelined squares/sqrts.
    warm = const.tile([128, 1], _F32, tag="warm")
    nc.scalar.activation(
        warm[:], ones[:, 0:1], mybir.ActivationFunctionType.Square
    )
    nc.scalar.sqrt(warm[:], warm[:])

    cj = 0  # global chunk id
    prev_pts = []  # previous block's PSUM round tiles (for PE warm-ups)
    for i, xt in enumerate(xts):
        for b in range(CB):
            xb16 = x16p.tile([128, P_TOT], _BF16, tag="xb16")
            parts = stat.tile([128, NCHUNK], _F32, tag="parts")
            for j in range(NCHUNK):
                stg = stage.tile([128, CHUNK_F], _F32, tag="stg")
                dma_eng = nc.scalar if cj % 3 == 1 else nc.sync
                dma_eng.dma_start(
                    stg[:], xt[b * 128 : (b + 1) * 128, bass.ts(j, CHUNK_F)]
                )
                # PE warm-up: one tiny matmul per chunk, gated on the
                # chunk's DMA (reads one staged cell), into a dead cell
                # (row 64, never drained) of a prior round tile.  Keeps
                # the HAM activity window busy so the real matmul bursts
                # run at 2.4 GHz instead of ramping from 1.2.
                if prev_pts:
                    wt = prev_pts[-2 + (j % 2)]
                    nc.tensor.matmul(
                        wt[64:65, 0, 0:1], ones[0:1, 0:1], stg[0:1, 0:1],
                        start=True, stop=True, skip_group_check=True,
                    )
                # fp32 -> bf16 for the TensorE stream
                x16c = xb16[:, bass.ts(j, CHUNK_F)]
                if j in CONV_VECTOR:
                    nc.vector.tensor_copy(x16c, stg[:])
                else:
                    nc.gpsimd.tensor_copy(x16c, stg[:])
                # column sum-of-squares partial for this chunk
                if j in SSQ_VECTOR:
                    sq = sqv.tile([128, CHUNK_F], _BF16, tag="sq")
                    nc.vector.scalar_tensor_tensor(
                        sq[:],
                        x16c,
                        1.0,
                        x16c,
                        op0=mybir.AluOpType.mult,
                        op1=mybir.AluOpType.mult,
                        accum_out=parts[:, j : j + 1],
                    )
                else:
                    sq = sqs.tile([128, CHUNK_F], _F32, tag="sq")
                    nc.scalar.activation(
                        sq[:],
                        stg[:],
                        mybir.ActivationFunctionType.Square,
                        accum_out=parts[:, j : j + 1],
                    )
                cj += 1

            # r = 1/sqrt(ssq): ACT-table sqrt (<=2ULP) + DVE reciprocal +
            # one Newton step on the reciprocal (y <- y*(2 - n*y)).
            ssq = stat.tile([128, 1], _F32, tag="ssq")
            nc.vector.reduce_sum(ssq[:], parts[:], axis=mybir.AxisListType.X)
            n_ = stat.tile([128, 1], _F32, tag="n_")
            nc.scalar.sqrt(n_[:], ssq[:])
            y = stat.tile([128, 1], _F32, tag="y")
            nc.vector.reciprocal(y[:], n_[:])
            t0 = stat.tile([128, 1], _F32, tag="t0")
            nc.vector.tensor_mul(t0[:], y[:], n_[:])
            t1 = stat.tile([128, 1], _F32, tag="t1")
            nc.vector.tensor_scalar(
                t1[:], t0[:], -1.0, 2.0,
                op0=mybir.AluOpType.mult, op1=mybir.AluOpType.add,
            )
            y2 = stat.tile([128, 1], _F32, tag="y2")
            nc.vector.tensor_mul(y2[:], y[:], t1[:])
            # replicate r across 32 stationary weight columns, in bf16
            yb = stat.tile([128, 32], _BF16, tag="yb")
            nc.vector.tensor_scalar(
                yb[:], ones[:], y2[:, 0:1], None, op0=mybir.AluOpType.mult
            )

            # s contributions: contract the 128 columns (partitions) via
            # matmul into 4-round ping-pong PSUM tiles (2 bases x 4
            # banks, 8 pixel chunks each).  Rows 0-31 replicate the
            # base-0 chunk, row 32 is the base-32 chunk's first replica,
            # so one [33, 2048] copy (cost = free size only) captures a
            # whole round; drains run on ScalarE and chase the matmul
            # stream.  Out rows 31:33 per round; host adds the
            # per-block partials.
            cur_pts = []
            for r in range(4):
                pt = psum.tile([128, 4, MMN], _F32, tag="pt")
                cur_pts.append(pt)
                for base_idx in range(2):
                    for bank in range(4):
                        j2 = r * 8 + base_idx * 4 + bank
                        nc.tensor.matmul(
                            pt[32 * base_idx : 32 * base_idx + 32, bank, :],
                            yb[:],
                            xb16[:, bass.ts(j2, MMN)],
                            start=True,
                            stop=True,
                        )
                st = stp.tile([33, 4 * MMN], _F32, tag="st")
                nc.vector.tensor_copy(st[:], pt[0:33, :, :])
                nc.sync.dma_start(s_out[i, b, r], st[31:33, :])
            prev_pts = cur_pts


def _hoist_excess_waits(nc):
    """Walrus rejects instructions whose encodings lack room for multiple
    semaphore waits (Activation/LoadWeights/DMA-direct2d allow just one).
    Hoist all-but-one wait of any instruction into standalone
    InstEventSemaphore waits on the same engine queue — semantically
    identical (the queue blocks at the event-sem instead)."""
    cnt = 0
    for f in nc.m.functions:
        for blk in f.blocks:
            insts = blk.instructions
            out = []
            changed = False
            for inst in insts:
                si = getattr(inst, "sync_info", None)
                waits = list(si.on_wait) if si is not None and si.on_wait else []
                if len(waits) > 1:
                    for w in waits[:-1]:
                        ev = mybir.InstEventSemaphore(
                            name=f"I-hoistw-{cnt}", ins=[], outs=[]
                        )
                        cnt += 1
                        ev.engine = inst.engine
                        ev.sync_info = mybir.SyncInfo(on_wait=[w], on_update=[])
                        out.append(ev)
                    inst.sync_info = mybir.SyncInfo(
                        on_wait=[waits[-1]],
                        on_update=list(si.on_update or []),
                    )
                    changed = True
                out.append(inst)
            if changed:
                insts[:] = out
    return cnt


def _build(hoist=True):
    # hoist=False is for CoreSim-based validation only (the simulator
    # can't ingest the raw-inserted event-sem instructions).
    key = ("nc", hoist)
    if key in _cache:
        return _cache[key]
    nc = bass.Bass("TRN2", target_bir_lowering=False, debug=False,
                   num_devices=N_CORES)
    x1t = nc.dram_tensor("x1t", [C_CORE, P_TOT], _F32, kind="ExternalInput").ap()
    x2t = nc.dram_tensor("x2t", [C_CORE, P_TOT], _F32, kind="ExternalInput").ap()
    s_out = nc.dram_tensor(
        "s_out", [2, CB, 4, 2, C_TOT], _F32, kind="ExternalOutput"
    ).ap()
    from contextlib import ExitStack

    with tile.TileContext(nc) as tc:
        with ExitStack() as ctx:
            _emit_core_kernel(nc, tc, ctx, [x1t, x2t], s_out)
    if hoist:
        _hoist_excess_waits(nc)
    _cache[key] = nc
    return nc


def _shard_inputs(input1, input2):
    """Column-shard + transpose: core k gets x[:, k*256:(k+1)*256].T
    contiguous [C_CORE, P_TOT] so DMA rows are 64 KiB contiguous."""
    in_maps = [{} for _ in range(N_CORES)]
    for name, arr in (("x1t", input1), ("x2t", input2)):
        x = np.ascontiguousarray(np.asarray(arr, dtype=np.float32)).reshape(
            P_TOT, C_TOT
        )
        xs = np.ascontiguousarray(x.reshape(P_TOT, N_CORES, C_CORE).transpose(1, 2, 0))
        for k in range(N_CORES):
            in_maps[k][name] = xs[k]
    return in_maps


def _unscramble(s_core):
    """s_core: [CB, 4, 2, 2048] for one input. Pixel index is
    (r*8 + base_idx*4 + bank)*512 + n = row-major flatten of
    [r, base_idx, bank, n]; block partials sum."""
    return s_core.astype(np.float64).sum(axis=0).reshape(P_TOT)


def kernel(input1, input2, _trace=False):
    global LAST_RESULTS
    nc = _build()
    in_maps = _shard_inputs(input1, input2)
    res = bass_utils.run_bass_kernel_spmd(
        nc, in_maps, core_ids=list(range(N_CORES)), trace=_trace,
    )
    LAST_RESULTS = res
    s1 = np.zeros(P_TOT, dtype=np.float64)
    s2 = np.zeros(P_TOT, dtype=np.float64)
    for r in res.results:
        so = r["s_out"]  # [2, CB, 4, 2, 2048]
        s1 += _unscramble(so[0])
        s2 += _unscramble(so[1])
    dot = float(np.dot(s1, s2))
    mean = dot / (C_TOT * C_TOT)
    return np.array(mean * mean, dtype=np.float32)


# revision 35
# speedup vs baseline: 1.9019x; 1.0328x over previous
"""Trainium2 Bass kernel for nn_Channel_Wise_DiffLoss.

Reference computation (P = 16384 pixels, C = 2048 columns = B*C_ch):
    x1 = input1.reshape(P, C);  x2 = input2.reshape(P, C)
    n_i[c] = sqrt(sum_p x_i[p,c]^2)          (per-column L2 norm)
    x_in = x_i / (n_i + 1e-6)
    out  = mean(x1n^T @ x2n) ** 2

Algebraic rewrite (no Gram matrix needed):
    mean(gram) = (1/C^2) * sum_p s1[p] * s2[p]
    where s_i[p] = sum_c x_i[p,c] * r_i[c],  r_i[c] = 1/(n_i[c] + 1e-6)

With 16384-element Gaussian columns, n ~ 128 >> 1e-6, and (n + 1e-6)
rounds to n exactly in fp32, so r = rsqrt(ssq) is exact.

Sharding: columns across the 8 cores (256 columns each). Column norms are
then fully core-local (each core holds the full pixel extent of its
columns) -> no collectives. Each core returns its partial s1/s2 vectors
(sum over its 256 columns); the host adds the 8 partials and does the
final tiny dot product.

Per-core pipeline (v2 — single pass over HBM, every engine under the
DMA roofline of ~94 us for the 32 MiB shard):
  - 32 chunks of [128 cols, 2048 pixels] fp32 (1 MiB each), DMA'd on BOTH
    HWDGE rings (sync + scalar engine issue) to aggregate queue bandwidth.
  - DVE converts each chunk fp32 -> bf16 (2x_2P mode, ~1.1 us/chunk) into
    a persistent per-block [128, 16384] bf16 tile.
  - Column sum-of-squares per chunk via ACT Square+accumulator (ScalarE)
    or scalar_tensor_tensor+accum (GPSIMD), split to balance engines.
  - Per block: reduce partials, sqrt (ACT table, 2ULP), DVE reciprocal +
    one Newton step, replicate to bf16 weights [128, 32].
  - TensorE: 32 matmuls [128,512] bf16 per (input, block) (1 col/cycle).
    PSUM bases are limited to {0,32,64}, so chunks land in 4-round
    ping-pong tiles of 2 bases x 4 banks (the baseline geometry); rows
    31:33 of each round drain to SBUF (split DVE/ScalarE) and DMA out.
    The host adds the two block partials per input.
"""

import numpy as np

import concourse.bass as bass
import concourse.mybir as mybir
from concourse import tile
from concourse import bass_utils

P_TOT = 16384  # pixels (H*W)
C_TOT = 2048  # columns (B*C)
N_CORES = 8
C_CORE = C_TOT // N_CORES  # 256 columns per core
CB = C_CORE // 128  # 2 column blocks of 128 partitions
NCHUNK = 8  # DMA/compute chunks per block (1 MiB each)
CHUNK_F = P_TOT // NCHUNK  # 2048 pixels per chunk
MMN = 512  # matmul moving free size (one PSUM bank of fp32)
NMM = P_TOT // MMN  # 32 matmul chunks per block

_F32 = mybir.dt.float32
_BF16 = mybir.dt.bfloat16

# Per-block chunk roles, balanced from the v3 trace (DVE was 98.8us —
# scalar_tensor_tensor runs at 1x, 2285ns, and stalled the stage
# pipeline; tensor_tensor_reduce mis-encodes - "ISA wrong length"):
#   - converts fp32->bf16: all DVE CAST (~1.2us 2x_2P).  GPSIMD CAST
#     measured 7.6us/chunk AND its shared SBUF port throttled concurrent
#     DVE ops 2.4x — keep GPSIMD idle.
#   - ssq: all ScalarE ACT Square+accum from the fp32 stage (~1.9us +
#     0.3us accumulator read, parallel with the convert)
#   - all PSUM round drains on DVE (~2.3us)
# Balance: ScalarE ~87us, DVE ~88us, under the ~102us DMA span.
SSQ_VECTOR = ()  # chunks per block with DVE ssq (rest ScalarE)
CONV_VECTOR = tuple(range(NCHUNK))  # chunks with DVE convert (rest GPSIMD)

_cache = {}

# Results of the last device run (BassKernelResults); the test harness
# reads exec_time_ns off this after calling kernel(..., _trace=True).
LAST_RESULTS = None


def _emit_core_kernel(nc, tc, ctx, xts, s_out):
    """xts = [x1t, x2t] DRAM APs [C_CORE, P_TOT]; s_out [2, 4, NMM//4, MMN]."""
    stage = ctx.enter_context(tc.tile_pool(name="stage", bufs=8))
    x16p = ctx.enter_context(tc.tile_pool(name="x16", bufs=2))
    stat = ctx.enter_context(tc.tile_pool(name="stat", bufs=8))
    const = ctx.enter_context(tc.tile_pool(name="const", bufs=1))
    sqs = ctx.enter_context(tc.tile_pool(name="sqs", bufs=2))
    sqv = ctx.enter_context(tc.tile_pool(name="sqv", bufs=2))
    psum = ctx.enter_context(tc.tile_pool(name="psum", bufs=2, space="PSUM"))
    stp = ctx.enter_context(tc.tile_pool(name="stp", bufs=3))

    ones = const.tile([128, 32], _F32, tag="ones")
    nc.vector.memset(ones[:], 1.0)
    ones16 = const.tile([128, 32], _BF16, tag="ones16")
    nc.vector.memset(ones16[:], 1.0)

    # Warm-up: trigger ACT table loads at kernel start so they don't land
    # on the pipelined squares/sqrts.
    warm = const.tile([128, 1], _F32, tag="warm")
    nc.scalar.activation(
        warm[:], ones[:, 0:1], mybir.ActivationFunctionType.Square
    )
    nc.scalar.sqrt(warm[:], warm[:])

    # Software-pipelined emission: block k's rounds (matmuls + drains)
    # are emitted AFTER block k+1's chunk phase.  Engines execute their
    # queues in order, so this keeps block k's DVE drains (which wait on
    # its matmuls) from blocking block k+1's CASTs — in v5 that hazard
    # stalled the stage pool and opened ~24us of DMA gaps.  Out-DMAs go
    # on GPSIMD's SWDGE ring for the same reason (on sync they'd block
    # later input-DMA issues).

    def emit_chunks(i, xt, b, cj0):
        xb16 = x16p.tile([128, P_TOT], _BF16, tag="xb16")
        parts = stat.tile([128, NCHUNK], _F32, tag="parts")
        for j in range(NCHUNK):
            stg = stage.tile([128, CHUNK_F], _F32, tag="stg")
            dma_eng = nc.scalar if (cj0 + j) % 3 == 1 else nc.sync
            dma_eng.dma_start(
                stg[:], xt[b * 128 : (b + 1) * 128, bass.ts(j, CHUNK_F)]
            )
            # fp32 -> bf16 for the TensorE stream
            x16c = xb16[:, bass.ts(j, CHUNK_F)]
            if j in CONV_VECTOR:
                nc.vector.tensor_copy(x16c, stg[:])
            else:
                nc.gpsimd.tensor_copy(x16c, stg[:])
            # column sum-of-squares partial for this chunk
            if j in SSQ_VECTOR:
                sq = sqv.tile([128, CHUNK_F], _BF16, tag="sq")
                nc.vector.scalar_tensor_tensor(
                    sq[:],
                    x16c,
                    1.0,
                    x16c,
                    op0=mybir.AluOpType.mult,
                    op1=mybir.AluOpType.mult,
                    accum_out=parts[:, j : j + 1],
                )
            else:
                sq = sqs.tile([128, CHUNK_F], _F32, tag="sq")
                nc.scalar.activation(
                    sq[:],
                    stg[:],
                    mybir.ActivationFunctionType.Square,
                    accum_out=parts[:, j : j + 1],
                )

        # r = 1/sqrt(ssq): ACT-table sqrt (<=2ULP) + DVE reciprocal +
        # one Newton step on the reciprocal (y <- y*(2 - n*y)).
        ssq = stat.tile([128, 1], _F32, tag="ssq")
        nc.vector.reduce_sum(ssq[:], parts[:], axis=mybir.AxisListType.X)
        n_ = stat.tile([128, 1], _F32, tag="n_")
        nc.scalar.sqrt(n_[:], ssq[:])
        y = stat.tile([128, 1], _F32, tag="y")
        nc.vector.reciprocal(y[:], n_[:])
        t0 = stat.tile([128, 1], _F32, tag="t0")
        nc.vector.tensor_mul(t0[:], y[:], n_[:])
        t1 = stat.tile([128, 1], _F32, tag="t1")
        nc.vector.tensor_scalar(
            t1[:], t0[:], -1.0, 2.0,
            op0=mybir.AluOpType.mult, op1=mybir.AluOpType.add,
        )
        y2 = stat.tile([128, 1], _F32, tag="y2")
        nc.vector.tensor_mul(y2[:], y[:], t1[:])
        # replicate r across 32 stationary weight columns, in bf16
        yb = stat.tile([128, 32], _BF16, tag="yb")
        nc.vector.tensor_scalar(
            yb[:], ones[:], y2[:, 0:1], None, op0=mybir.AluOpType.mult
        )
        return xb16, yb

    def emit_warm(xb16_next, tiles):
        # PE clock warm-up: the HAM drops the PE to 1.2 GHz after ~3.4us
        # of idle, and a burst needs ~3.4us of sustained work to reach
        # 2.4 GHz.  Spend that ramp on junk matmuls (into dead rows
        # 64-95 of already-drained round tiles) gated on the NEXT
        # block's chunk-6 bf16 copy, so they run while its chunk 7 +
        # rsqrt finish and the real burst starts warm.  Emitted after
        # the current block's rounds so the PE queue order is
        # [real MMs(k-1)][warm(k)][real MMs(k)].
        for w in range(12):
            wt = tiles[w % len(tiles)]
            nc.tensor.matmul(
                wt[64:96, w % 4, :],
                ones16[:],
                xb16_next[:, bass.ts(24 + (w % 4), MMN)],
                start=True,
                stop=True,
                skip_group_check=True,
            )

    def emit_rounds(i, b, xb16, yb, xb16_next):
        # s contributions: contract the 128 columns (partitions) via
        # matmul into 4-round ping-pong PSUM tiles (2 bases x 4 banks, 8
        # pixel chunks each).  Rows 0-31 replicate the base-0 chunk, row
        # 32 is the base-32 chunk's first replica, so one [33, 2048]
        # copy (cost = free size only) captures a whole round; drains
        # chase the matmul stream.  Out rows 31:33 per round; host adds
        # the per-block partials.
        cur_pts = []
        for r in range(4):
            pt = psum.tile([128, 4, MMN], _F32, tag="pt")
            cur_pts.append(pt)
            for base_idx in range(2):
                for bank in range(4):
                    j2 = r * 8 + base_idx * 4 + bank
                    nc.tensor.matmul(
                        pt[32 * base_idx : 32 * base_idx + 32, bank, :],
                        yb[:],
                        xb16[:, bass.ts(j2, MMN)],
                        start=True,
                        stop=True,
                    )
            st = stp.tile([33, 4 * MMN], _F32, tag="st")
            nc.vector.tensor_copy(st[:], pt[0:33, :, :])
            nc.gpsimd.dma_start(s_out[i, b, r], st[31:33, :])
        if xb16_next is not None:
            emit_warm(xb16_next, cur_pts)

    blocks = [(i, xt, b) for i, xt in enumerate(xts) for b in range(CB)]
    pending = None  # (i, b, xb16, yb) awaiting round emission
    for k, (i, xt, b) in enumerate(blocks):
        xb16, yb = emit_chunks(i, xt, b, 8 * k)
        if k == 0:
            # block 0 has no prior round tiles; warm into two scratch
            # PSUM tiles (their slots are recycled by round 0's allocs,
            # which only touch rows 0-63 — the junk rows are dead).
            w0a = psum.tile([128, 4, MMN], _F32, tag="pt")
            w0b = psum.tile([128, 4, MMN], _F32, tag="pt")
            emit_warm(xb16, [w0a, w0b])
        if pending is not None:
            emit_rounds(*pending, xb16_next=xb16)
        pending = (i, b, xb16, yb)
    emit_rounds(*pending, xb16_next=None)


def _hoist_excess_waits(nc):
    """Walrus rejects instructions whose encodings lack room for multiple
    semaphore waits (Activation/LoadWeights/DMA-direct2d allow just one).
    Hoist all-but-one wait of any instruction into standalone
    InstEventSemaphore waits on the same engine queue — semantically
    identical (the queue blocks at the event-sem instead)."""
    cnt = 0
    for f in nc.m.functions:
        for blk in f.blocks:
            insts = blk.instructions
            out = []
            changed = False
            for inst in insts:
                si = getattr(inst, "sync_info", None)
                waits = list(si.on_wait) if si is not None and si.on_wait else []
                if len(waits) > 1:
                    for w in waits[:-1]:
                        ev = mybir.InstEventSemaphore(
                            name=f"I-hoistw-{cnt}", ins=[], outs=[]
                        )
                        cnt += 1
                        ev.engine = inst.engine
                        ev.sync_info = mybir.SyncInfo(on_wait=[w], on_update=[])
                        out.append(ev)
                    inst.sync_info = mybir.SyncInfo(
                        on_wait=[waits[-1]],
                        on_update=list(si.on_update or []),
                    )
                    changed = True
                out.append(inst)
            if changed:
                insts[:] = out
    return cnt


def _build(hoist=True):
    # hoist=False is for CoreSim-based validation only (the simulator
    # can't ingest the raw-inserted event-sem instructions).
    key = ("nc", hoist)
    if key in _cache:
        return _cache[key]
    nc = bass.Bass("TRN2", target_bir_lowering=False, debug=False,
                   num_devices=N_CORES)
    x1t = nc.dram_tensor("x1t", [C_CORE, P_TOT], _F32, kind="ExternalInput").ap()
    x2t = nc.dram_tensor("x2t", [C_CORE, P_TOT], _F32, kind="ExternalInput").ap()
    s_out = nc.dram_tensor(
        "s_out", [2, CB, 4, 2, C_TOT], _F32, kind="ExternalOutput"
    ).ap()
    from contextlib import ExitStack

    with tile.TileContext(nc) as tc:
        with ExitStack() as ctx:
            _emit_core_kernel(nc, tc, ctx, [x1t, x2t], s_out)
    if hoist:
        _hoist_excess_waits(nc)
    _cache[key] = nc
    return nc


def _shard_inputs(input1, input2):
    """Column-shard + transpose: core k gets x[:, k*256:(k+1)*256].T
    contiguous [C_CORE, P_TOT] so DMA rows are 64 KiB contiguous."""
    in_maps = [{} for _ in range(N_CORES)]
    for name, arr in (("x1t", input1), ("x2t", input2)):
        x = np.ascontiguousarray(np.asarray(arr, dtype=np.float32)).reshape(
            P_TOT, C_TOT
        )
        xs = np.ascontiguousarray(x.reshape(P_TOT, N_CORES, C_CORE).transpose(1, 2, 0))
        for k in range(N_CORES):
            in_maps[k][name] = xs[k]
    return in_maps


def _unscramble(s_core):
    """s_core: [CB, 4, 2, 2048] for one input. Pixel index is
    (r*8 + base_idx*4 + bank)*512 + n = row-major flatten of
    [r, base_idx, bank, n]; block partials sum."""
    return s_core.astype(np.float64).sum(axis=0).reshape(P_TOT)


def kernel(input1, input2, _trace=False):
    global LAST_RESULTS
    nc = _build()
    in_maps = _shard_inputs(input1, input2)
    res = bass_utils.run_bass_kernel_spmd(
        nc, in_maps, core_ids=list(range(N_CORES)), trace=_trace,
    )
    LAST_RESULTS = res
    s1 = np.zeros(P_TOT, dtype=np.float64)
    s2 = np.zeros(P_TOT, dtype=np.float64)
    for r in res.results:
        so = r["s_out"]  # [2, CB, 4, 2, 2048]
        s1 += _unscramble(so[0])
        s2 += _unscramble(so[1])
    dot = float(np.dot(s1, s2))
    mean = dot / (C_TOT * C_TOT)
    return np.array(mean * mean, dtype=np.float32)
